# revision 9
# baseline (speedup 1.0000x reference)
"""CPAttention TRN2 kernel v5: v4 + PSUM-free pair tails, split loads,
carry software-pipelining.

  - QKV q/k f32r, V bf16, dots f32r (2-head row packing)
  - t = dots*mask -> fp16 (Vector), es = exp bf16 (Scalar, [128,2048])
  - |t| fp16 via u16 bitand (Vector, 4x mode)
  - AV bf16 ones-augmented (Z free at row 64); score strips fp16 at row 96
  - pair tail: PSUM extracted via GpSimd copies (o), Scalar (Z, strips);
    1/Z via Vector recip on shifted row; GpSimd partition_broadcast;
    onorm all-bf16 2-byte multiply
  - outproj bf16 K=128 (head-B half DMA-moved to rows 64:128)
  - host: bias add, strip-sum fp64, argsort, swap
"""
import numpy as np

import concourse.bacc as bacc
import concourse.tile as tile
from concourse import mybir
from concourse.bass_utils import run_bass_kernel_spmd

F32 = mybir.dt.float32
F32R = mybir.dt.float32r
BF16 = mybir.dt.bfloat16
F16 = mybir.dt.float16
U16 = mybir.dt.uint16
AOP = mybir.AluOpType
AFT = mybir.ActivationFunctionType

B, N, DIM = 8, 1024, 512
HEADS, DH = 8, 64
INNER = 512
SCALE = DH ** -0.5

_cache = {}


def _build():
    nc = bacc.Bacc()
    xT = nc.declare_dram_parameter("xT", [DIM, N], F32R, isOutput=False)
    xTbf = nc.declare_dram_parameter("xTbf", [DIM, N], BF16, isOutput=False)
    maskT = nc.declare_dram_parameter("maskT", [N, N], BF16, isOutput=False)
    wqk = nc.declare_dram_parameter("wqk", [DIM, 2 * INNER], F32R, isOutput=False)
    wvbf = nc.declare_dram_parameter("wvbf", [DIM, INNER], BF16, isOutput=False)
    wobf = nc.declare_dram_parameter("wobf", [INNER, DIM], BF16, isOutput=False)
    y_out = nc.declare_dram_parameter("y", [N, DIM], F32, isOutput=True)
    sc_out = nc.declare_dram_parameter("score", [8, N], F32, isOutput=True)
    zscr = nc.declare_dram_parameter("zscr", [8, N], BF16, isOutput=True)

    with tile.TileContext(nc) as tc:
        with tc.tile_pool(name="cst", bufs=1) as cst, \
             tc.tile_pool(name="wt", bufs=4) as wt, \
             tc.tile_pool(name="wes", bufs=4) as wes, \
             tc.tile_pool(name="wq2", bufs=2) as wq2, \
             tc.tile_pool(name="wyt", bufs=2) as wyt, \
             tc.tile_pool(name="war", bufs=4) as war, \
             tc.tile_pool(name="wz", bufs=1) as wz, \
             tc.tile_pool(name="ppA", bufs=1, space="PSUM") as ppA, \
             tc.tile_pool(name="ppB", bufs=1, space="PSUM") as ppB, \
             tc.tile_pool(name="ppT", bufs=1, space="PSUM") as ppT, \
             tc.tile_pool(name="ppU", bufs=1, space="PSUM") as ppU:

            # ---- loads (split per kt so QKV starts early) ----
            xt = cst.tile([128, 4, N], F32R)
            wq = cst.tile([128, 4, 2 * INNER], F32R)
            for kt in range(4):
                nc.sync.dma_start(
                    out=xt[:, kt, :], in_=xT[kt * 128:(kt + 1) * 128, :])
                nc.scalar.dma_start(
                    out=wq[:, kt, :], in_=wqk[kt * 128:(kt + 1) * 128, :])
            xtb = cst.tile([128, 4, N], BF16)
            nc.sync.dma_start(out=xtb, in_=xTbf[:, :].rearrange("(t p) i -> p t i", p=128))
            wvb = cst.tile([128, 4, INNER], BF16)
            nc.sync.dma_start(out=wvb, in_=wvbf[:, :].rearrange("(t p) c -> p t c", p=128))
            msk = cst.tile([128, 8, N], BF16)
            nc.sync.dma_start(out=msk, in_=maskT[:, :].rearrange("(t p) i -> p t i", p=128))
            wob = cst.tile([128, 4, DIM], BF16)
            nc.sync.dma_start(out=wob, in_=wobf[:, :].rearrange("(t p) e -> p t e", p=128))

            ones1r = cst.tile([128, 1], F16)
            nc.vector.memset(ones1r, 1.0)

            qkT = cst.tile([128, 8, N], F32R)
            vv = cst.tile([128, HEADS, 8, 65], BF16)
            nc.vector.memset(vv[:, :, :, 64:65], 1.0)
            onorm = cst.tile([128, 4, N], BF16)

            # ---- QKV q/k (f32r) ----
            for ct in range(8):
                pool = ppT if ct % 2 == 0 else ppU
                pq = pool.tile([128, N], F32, tag="t" if ct % 2 == 0 else "u")
                for ic in range(2):
                    sl = slice(ic * 512, (ic + 1) * 512)
                    for kt in range(4):
                        nc.tensor.matmul(
                            pq[:, sl],
                            wq[:, kt, ct * 128:(ct + 1) * 128],
                            xt[:, kt, sl],
                            start=(kt == 0), stop=(kt == 3))
                if ct % 2 == 0:
                    nc.scalar.activation(out=qkT[:, ct, :], in_=pq, func=AFT.Copy)
                else:
                    nc.vector.tensor_copy(qkT[:, ct, :], pq)

            # ---- V (bf16) + ones augment ----
            for jt in range(8):
                pool = ppT if jt % 2 == 0 else ppU
                pv = pool.tile([128, N], F32, tag="t" if jt % 2 == 0 else "u")
                for kt in range(4):
                    nc.tensor.matmul(
                        pv[:, 0:512],
                        xtb[:, kt, jt * 128:(jt + 1) * 128],
                        wvb[:, kt, :],
                        start=(kt == 0), stop=(kt == 3))
                nc.vector.tensor_copy(
                    vv[:, :, jt, 0:64],
                    pv[:, 0:512].rearrange("p (h d) -> p h d", h=HEADS))

            # ---- attention (carry-pipelined) ----
            def emit_back(TA, TB, hA, hB, jt, es2, ab2, first, last):
                for ic in range(2):
                    sl = slice(ic * 512, (ic + 1) * 512)
                    nc.tensor.matmul(
                        TA[0:65, sl], vv[:, hA, jt, :], es2[:, 0, sl],
                        start=first, stop=last,
                        tile_position=(0, 0), skip_group_check=True)
                    nc.tensor.matmul(
                        TB[0:65, sl], vv[:, hB, jt, :], es2[:, 1, sl],
                        start=first, stop=last,
                        tile_position=(0, 0), skip_group_check=True)
                    nc.tensor.matmul(
                        TA[96:97, sl], ones1r, ab2[:, 0, sl],
                        start=first, stop=last,
                        tile_position=(0, 96), skip_group_check=True)
                    nc.tensor.matmul(
                        TB[96:97, sl], ones1r, ab2[:, 1, sl],
                        start=first, stop=last,
                        tile_position=(0, 96), skip_group_check=True)

            for pr in range(4):
                hA, hB = 2 * pr, 2 * pr + 1
                TA = ppT.tile([128, N], F32, tag="t")
                TB = ppU.tile([128, N], F32, tag="u")
                carry = None
                for jt in range(8):
                    dts = []
                    for ic in range(2):
                        sl = slice(ic * 512, (ic + 1) * 512)
                        dA = ppA.tile([128, 512], F32, tag=f"a{ic}")
                        dB = ppB.tile([128, 512], F32, tag=f"b{ic}")
                        nc.tensor.matmul(
                            dA,
                            qkT[0:64, 4 + pr, jt * 128:(jt + 1) * 128],
                            qkT[0:64, pr, sl],
                            start=True, stop=True, tile_position=(0, 0))
                        nc.tensor.matmul(
                            dB,
                            qkT[64:128, 4 + pr, jt * 128:(jt + 1) * 128],
                            qkT[64:128, pr, sl],
                            start=True, stop=True, tile_position=(64, 0))
                        dts.append((dA, dB))
                    if carry is not None:
                        emit_back(TA, TB, hA, hB, *carry,
                                  first=(carry[0] == 0), last=False)
                    t2 = wt.tile([128, 2, N], F16, tag="t2")
                    for ic in range(2):
                        sl = slice(ic * 512, (ic + 1) * 512)
                        nc.vector.tensor_tensor(out=t2[:, 0, sl], in0=dts[ic][0],
                                                in1=msk[:, jt, sl], op=AOP.mult)
                        nc.vector.tensor_tensor(out=t2[:, 1, sl], in0=dts[ic][1],
                                                in1=msk[:, jt, sl], op=AOP.mult)
                    es2 = wes.tile([128, 2, N], BF16, tag="es")
                    nc.scalar.activation(out=es2, in_=t2, func=AFT.Exp, scale=SCALE)
                    ab2 = war.tile([128, 2, N], F16, tag="ab")
                    nc.vector.tensor_scalar(
                        out=ab2.bitcast(U16), in0=t2.bitcast(U16),
                        scalar1=0x7FFF, scalar2=None,
                        op0=AOP.bitwise_and)
                    carry = (jt, es2, ab2)
                emit_back(TA, TB, hA, hB, *carry, first=False, last=True)

                # ---- pair tail (PSUM-free: extract, then release TA/TB) ----
                oAs = wz.tile([64, N], BF16, tag="oa")
                oBs = wz.tile([64, N], BF16, tag="ob")
                nc.vector.tensor_copy(oAs, TA[0:64, :])
                nc.scalar.activation(out=oBs, in_=TB[0:64, :], func=AFT.Copy)
                zc = wz.tile([128, 2, N], F32, tag="zc")
                nc.scalar.activation(out=zc[64:65, 0, :], in_=TA[64:65, :], func=AFT.Copy)
                nc.scalar.activation(out=zc[64:65, 1, :], in_=TB[64:65, :], func=AFT.Copy)
                scs = wz.tile([128, 2, N], F32, tag="sc")
                nc.scalar.activation(out=scs[96:97, 0, :], in_=TA[96:97, :], func=AFT.Copy)
                nc.scalar.activation(out=scs[96:97, 1, :], in_=TB[96:97, :], func=AFT.Copy)
                nc.sync.dma_start(out=sc_out[2 * pr:2 * pr + 2, :], in_=scs[96:97, :, :])
                # 1/Z: DMA-shift p64/p64 -> p0/p1, fp32 recip (2 lanes),
                # bf16 convert, stride-0 broadcast DMA over 64 partitions
                zc0 = wz.tile([2, N], F32, tag="z0")
                nc.sync.dma_start(out=zc0[0:1, :], in_=zc[64:65, 0, :])
                nc.sync.dma_start(out=zc0[1:2, :], in_=zc[64:65, 1, :])
                zrf = wz.tile([2, N], F32, tag="zf")
                nc.vector.reciprocal_approx_fast(out=zrf, in_=zc0)
                zrb = wz.tile([2, N], BF16, tag="zb")
                nc.vector.tensor_copy(zrb, zrf)
                nc.sync.dma_start(out=zscr[2 * pr:2 * pr + 2, :], in_=zrb)
                zbs = wz.tile([64, 2, N], BF16, tag="zs")
                nc.sync.dma_start(
                    out=zbs[:, 0, :],
                    in_=zscr[2 * pr:2 * pr + 1, :].to_broadcast([64, N]))
                nc.sync.dma_start(
                    out=zbs[:, 1, :],
                    in_=zscr[2 * pr + 1:2 * pr + 2, :].to_broadcast([64, N]))
                # onorm = o * (1/Z)  (all 2-byte)
                nc.vector.tensor_tensor(out=onorm[0:64, pr, :], in0=oAs,
                                        in1=zbs[:, 0, :], op=AOP.mult)
                onb = wz.tile([64, N], BF16, tag="on")
                nc.vector.tensor_tensor(out=onb, in0=oBs,
                                        in1=zbs[:, 1, :], op=AOP.mult)
                nc.sync.dma_start(out=onorm[64:128, pr, :], in_=onb)

            # ---- output projection (bf16, K=128 per pair) ----
            ytags = ["a0", "b0", "a1", "b1"]
            for it in range(8):
                pool = [ppA, ppB, ppA, ppB][it % 4]
                yp = pool.tile([128, 512], F32, tag=ytags[it % 4])
                for pr in range(4):
                    nc.tensor.matmul(
                        yp,
                        onorm[:, pr, it * 128:(it + 1) * 128],
                        wob[:, pr, :],
                        start=(pr == 0), stop=(pr == 3))
                yt = wyt.tile([128, DIM], F32, tag="yt")
                if it % 2 == 0:
                    nc.vector.tensor_copy(yt, yp)
                else:
                    nc.scalar.activation(out=yt, in_=yp, func=AFT.Copy)
                if it % 2 == 0:
                    nc.sync.dma_start(out=y_out[it * 128:(it + 1) * 128, :], in_=yt)
                else:
                    nc.scalar.dma_start(out=y_out[it * 128:(it + 1) * 128, :], in_=yt)
    nc.finalize()
    return nc


def _get_nc():
    if "nc" not in _cache:
        _cache["nc"] = _build()
    return _cache["nc"]


def _run_device(inputs, trace=False):
    _cache["b_out"] = np.asarray(inputs["b_out"], np.float32)
    x = np.asarray(inputs["x"], np.float32)
    cp_mask = np.asarray(inputs["cp_mask"])
    w_qkv = np.asarray(inputs["w_qkv"], np.float32)
    w_out = np.asarray(inputs["w_out"], np.float32)

    bf = mybir.dt.np(BF16)
    maskT = np.ascontiguousarray(cp_mask.T).astype(bf)
    wqk = np.ascontiguousarray(w_qkv[:, :2 * INNER])
    wvbf = np.ascontiguousarray(w_qkv[:, 2 * INNER:]).astype(bf)
    wobf = np.ascontiguousarray(w_out).astype(bf)

    in_maps = []
    for b in range(B):
        xTb = np.ascontiguousarray(x[b].T)
        in_maps.append({
            "xT": xTb,
            "xTbf": xTb.astype(bf),
            "maskT": maskT,
            "wqk": wqk,
            "wvbf": wvbf,
            "wobf": wobf,
        })

    nc = _get_nc()
    res = run_bass_kernel_spmd(nc, in_maps, core_ids=list(range(B)), trace=trace)
    y = np.stack([res.results[b]["y"] for b in range(B)])
    strips = np.stack([res.results[b]["score"] for b in range(B)])  # [B, 8, N]
    nnz = np.count_nonzero(cp_mask, axis=1)
    score = strips.astype(np.float64).sum(axis=1) * SCALE / nnz[None, :]
    return y, score.astype(np.float32), res


def _apply_swap(y, score, patches):
    b_out = _cache.get("b_out")
    if b_out is not None:
        y = y + b_out
    idx = np.argsort(score, axis=-1, kind="stable")[::-1]
    out = y.copy()
    clone = y
    bi = np.arange(B)
    for i in range(1, patches + 1):
        ti = idx[:, i]
        out[bi, i] = clone[bi, ti]
        out[bi, ti] = clone[:, i]
    return out


def kernel(**inputs):
    patches = int(np.asarray(inputs["patches_in_core_nodes"]))
    y, score, _ = _run_device(inputs, trace=False)
    return _apply_swap(y, score, patches)


# revision 11
# speedup vs baseline: 1.2025x; 1.2025x over previous
"""CPAttention TRN2 kernel v5: v4 + PSUM-free pair tails, split loads,
carry software-pipelining.

  - QKV q/k f32r, V bf16, dots f32r (2-head row packing)
  - t = dots*mask -> fp16 (Vector), es = exp bf16 (Scalar, [128,2048])
  - |t| fp16 via u16 bitand (Vector, 4x mode)
  - AV bf16 ones-augmented (Z free at row 64); score strips fp16 at row 96
  - pair tail: PSUM extracted via GpSimd copies (o), Scalar (Z, strips);
    1/Z via Vector recip on shifted row; GpSimd partition_broadcast;
    onorm all-bf16 2-byte multiply
  - outproj bf16 K=128 (head-B half DMA-moved to rows 64:128)
  - host: bias add, strip-sum fp64, argsort, swap
"""
import numpy as np

import concourse.bacc as bacc
import concourse.tile as tile
from concourse import mybir
from concourse.bass_utils import run_bass_kernel_spmd

F32 = mybir.dt.float32
F32R = mybir.dt.float32r
BF16 = mybir.dt.bfloat16
F16 = mybir.dt.float16
U16 = mybir.dt.uint16
AOP = mybir.AluOpType
AFT = mybir.ActivationFunctionType

B, N, DIM = 8, 1024, 512
HEADS, DH = 8, 64
INNER = 512
SCALE = DH ** -0.5

_cache = {}


def _build():
    nc = bacc.Bacc()
    xT = nc.declare_dram_parameter("xT", [DIM, N], F32R, isOutput=False)
    xTbf = nc.declare_dram_parameter("xTbf", [DIM, N], BF16, isOutput=False)
    maskT = nc.declare_dram_parameter("maskT", [N, N], BF16, isOutput=False)
    wqk = nc.declare_dram_parameter("wqk", [DIM, 2 * INNER], F32R, isOutput=False)
    wvbf = nc.declare_dram_parameter("wvbf", [DIM, INNER], BF16, isOutput=False)
    wobf = nc.declare_dram_parameter("wobf", [INNER, DIM], BF16, isOutput=False)
    y_out = nc.declare_dram_parameter("y", [N, DIM], F32, isOutput=True)
    sc_out = nc.declare_dram_parameter("score", [8, N], F32, isOutput=True)
    zscr = nc.declare_dram_parameter("zscr", [8, N], BF16, isOutput=True)

    with tile.TileContext(nc) as tc:
        with tc.tile_pool(name="cst", bufs=1) as cst, \
             tc.tile_pool(name="wt", bufs=3) as wt, \
             tc.tile_pool(name="wes", bufs=3) as wes, \
             tc.tile_pool(name="wq2", bufs=2) as wq2, \
             tc.tile_pool(name="wyt", bufs=3) as wyt, \
             tc.tile_pool(name="war", bufs=3) as war, \
             tc.tile_pool(name="wz", bufs=1) as wz, \
             tc.tile_pool(name="ppA", bufs=1, space="PSUM") as ppA, \
             tc.tile_pool(name="ppB", bufs=1, space="PSUM") as ppB, \
             tc.tile_pool(name="ppT", bufs=1, space="PSUM") as ppT, \
             tc.tile_pool(name="ppU", bufs=1, space="PSUM") as ppU:

            # ---- loads (split per kt so QKV starts early) ----
            xt = cst.tile([128, 4, N], F32R)
            wq = cst.tile([128, 4, 2 * INNER], F32R)
            for kt in range(4):
                nc.sync.dma_start(
                    out=xt[:, kt, :], in_=xT[kt * 128:(kt + 1) * 128, :])
                nc.scalar.dma_start(
                    out=wq[:, kt, :], in_=wqk[kt * 128:(kt + 1) * 128, :])
            xtb = cst.tile([128, 4, N], BF16)
            nc.sync.dma_start(out=xtb, in_=xTbf[:, :].rearrange("(t p) i -> p t i", p=128))
            wvb = cst.tile([128, 4, INNER], BF16)
            nc.sync.dma_start(out=wvb, in_=wvbf[:, :].rearrange("(t p) c -> p t c", p=128))
            msk = cst.tile([128, 8, N], BF16)
            nc.sync.dma_start(out=msk, in_=maskT[:, :].rearrange("(t p) i -> p t i", p=128))
            wob = cst.tile([128, 4, DIM], BF16)
            nc.sync.dma_start(out=wob, in_=wobf[:, :].rearrange("(t p) e -> p t e", p=128))

            ones1r = cst.tile([128, 1], F16)
            nc.vector.memset(ones1r, 1.0)

            qkT = cst.tile([128, 8, N], F32R)
            vv = cst.tile([128, HEADS, 8, 65], BF16)
            nc.vector.memset(vv[:, :, :, 64:65], 1.0)
            onorm = cst.tile([128, 4, N], BF16)

            # ---- QKV q/k (f32r) ----
            for ct in range(8):
                pool = ppT if ct % 2 == 0 else ppU
                pq = pool.tile([128, N], F32, tag="t" if ct % 2 == 0 else "u")
                for ic in range(2):
                    sl = slice(ic * 512, (ic + 1) * 512)
                    for kt in range(4):
                        nc.tensor.matmul(
                            pq[:, sl],
                            wq[:, kt, ct * 128:(ct + 1) * 128],
                            xt[:, kt, sl],
                            start=(kt == 0), stop=(kt == 3))
                if ct % 2 == 0:
                    nc.scalar.activation(out=qkT[:, ct, :], in_=pq, func=AFT.Copy)
                else:
                    nc.vector.tensor_copy(qkT[:, ct, :], pq)

            # ---- V (bf16) + ones augment ----
            for jt in range(8):
                pool = ppT if jt % 2 == 0 else ppU
                pv = pool.tile([128, N], F32, tag="t" if jt % 2 == 0 else "u")
                for kt in range(4):
                    nc.tensor.matmul(
                        pv[:, 0:512],
                        xtb[:, kt, jt * 128:(jt + 1) * 128],
                        wvb[:, kt, :],
                        start=(kt == 0), stop=(kt == 3))
                nc.vector.tensor_copy(
                    vv[:, :, jt, 0:64],
                    pv[:, 0:512].rearrange("p (h d) -> p h d", h=HEADS))

            # ---- attention (carry-pipelined) ----
            def emit_back(TA, TB, hA, hB, jt, es2, ab2, first, last):
                for ic in range(2):
                    sl = slice(ic * 512, (ic + 1) * 512)
                    nc.tensor.matmul(
                        TA[0:65, sl], vv[:, hA, jt, :], es2[:, 0, sl],
                        start=first, stop=last,
                        tile_position=(0, 0), skip_group_check=True)
                    nc.tensor.matmul(
                        TB[0:65, sl], vv[:, hB, jt, :], es2[:, 1, sl],
                        start=first, stop=last,
                        tile_position=(0, 0), skip_group_check=True)
                    nc.tensor.matmul(
                        TA[96:97, sl], ones1r, ab2[:, 0, sl],
                        start=first, stop=last,
                        tile_position=(0, 96), skip_group_check=True)
                    nc.tensor.matmul(
                        TB[96:97, sl], ones1r, ab2[:, 1, sl],
                        start=first, stop=last,
                        tile_position=(0, 96), skip_group_check=True)

            for pr in range(4):
                hA, hB = 2 * pr, 2 * pr + 1
                TA = ppT.tile([128, N], F32, tag="t")
                TB = ppU.tile([128, N], F32, tag="u")
                carry = None
                for jt in range(8):
                    dts = []
                    for ic in range(2):
                        sl = slice(ic * 512, (ic + 1) * 512)
                        dA = ppA.tile([128, 512], F32, tag=f"a{ic}")
                        dB = ppB.tile([128, 512], F32, tag=f"b{ic}")
                        nc.tensor.matmul(
                            dA,
                            qkT[0:64, 4 + pr, jt * 128:(jt + 1) * 128],
                            qkT[0:64, pr, sl],
                            start=True, stop=True, tile_position=(0, 0))
                        nc.tensor.matmul(
                            dB,
                            qkT[64:128, 4 + pr, jt * 128:(jt + 1) * 128],
                            qkT[64:128, pr, sl],
                            start=True, stop=True, tile_position=(64, 0))
                        dts.append((dA, dB))
                    if carry is not None:
                        emit_back(TA, TB, hA, hB, *carry,
                                  first=(carry[0] == 0), last=False)
                    t2 = wt.tile([128, 2, N], F16, tag="t2")
                    for ic in range(2):
                        sl = slice(ic * 512, (ic + 1) * 512)
                        nc.vector.tensor_tensor(out=t2[:, 0, sl], in0=dts[ic][0],
                                                in1=msk[:, jt, sl], op=AOP.mult)
                        nc.vector.tensor_tensor(out=t2[:, 1, sl], in0=dts[ic][1],
                                                in1=msk[:, jt, sl], op=AOP.mult)
                    es2 = wes.tile([128, 2, N], BF16, tag="es")
                    nc.scalar.activation(out=es2, in_=t2, func=AFT.Exp, scale=SCALE)
                    ab2 = war.tile([128, 2, N], F16, tag="ab")
                    nc.vector.tensor_scalar(
                        out=ab2.bitcast(U16), in0=t2.bitcast(U16),
                        scalar1=0x7FFF, scalar2=None,
                        op0=AOP.bitwise_and)
                    carry = (jt, es2, ab2)
                emit_back(TA, TB, hA, hB, *carry, first=False, last=True)

                # ---- pair tail (PSUM-free: extract, then release TA/TB) ----
                oAs = wz.tile([64, N], BF16, tag="oa")
                oBs = wz.tile([64, N], BF16, tag="ob")
                nc.vector.tensor_copy(oAs, TA[0:64, :])
                nc.scalar.activation(out=oBs, in_=TB[0:64, :], func=AFT.Copy)
                zc = wz.tile([128, 2, N], F32, tag="zc")
                nc.scalar.activation(out=zc[64:65, 0, :], in_=TA[64:65, :], func=AFT.Copy)
                nc.scalar.activation(out=zc[64:65, 1, :], in_=TB[64:65, :], func=AFT.Copy)
                scs = wz.tile([128, 2, N], F32, tag="sc")
                nc.scalar.activation(out=scs[96:97, 0, :], in_=TA[96:97, :], func=AFT.Copy)
                nc.scalar.activation(out=scs[96:97, 1, :], in_=TB[96:97, :], func=AFT.Copy)
                nc.sync.dma_start(out=sc_out[2 * pr:2 * pr + 2, :], in_=scs[96:97, :, :])
                # 1/Z: DMA-shift p64/p64 -> p0/p1, fp32 recip (2 lanes),
                # bf16 convert, stride-0 broadcast DMA over 64 partitions
                zc0 = wz.tile([2, N], F32, tag="z0")
                nc.sync.dma_start(out=zc0[0:1, :], in_=zc[64:65, 0, :])
                nc.sync.dma_start(out=zc0[1:2, :], in_=zc[64:65, 1, :])
                zrf = wz.tile([2, N], F32, tag="zf")
                nc.vector.reciprocal_approx_fast(out=zrf, in_=zc0)
                zrb = wz.tile([2, N], BF16, tag="zb")
                nc.vector.tensor_copy(zrb, zrf)
                nc.sync.dma_start(out=zscr[2 * pr:2 * pr + 2, :], in_=zrb)
                zbs = wz.tile([64, 2, N], BF16, tag="zs")
                nc.sync.dma_start(
                    out=zbs[:, 0, :],
                    in_=zscr[2 * pr:2 * pr + 1, :].to_broadcast([64, N]))
                nc.sync.dma_start(
                    out=zbs[:, 1, :],
                    in_=zscr[2 * pr + 1:2 * pr + 2, :].to_broadcast([64, N]))
                # onorm = o * (1/Z)  (all 2-byte)
                nc.vector.tensor_tensor(out=onorm[0:64, pr, :], in0=oAs,
                                        in1=zbs[:, 0, :], op=AOP.mult)
                onb = wz.tile([64, N], BF16, tag="on")
                nc.vector.tensor_tensor(out=onb, in0=oBs,
                                        in1=zbs[:, 1, :], op=AOP.mult)
                nc.sync.dma_start(out=onorm[64:128, pr, :], in_=onb)

            # ---- output projection (bf16, K=128 per pair) ----
            ytags = ["a0", "b0", "a1", "b1"]
            for it in range(8):
                pool = [ppA, ppB, ppA, ppB][it % 4]
                yp = pool.tile([128, 512], F32, tag=ytags[it % 4])
                for pr in range(4):
                    nc.tensor.matmul(
                        yp,
                        onorm[:, pr, it * 128:(it + 1) * 128],
                        wob[:, pr, :],
                        start=(pr == 0), stop=(pr == 3))
                yt = wyt.tile([128, DIM], F32, tag="yt")
                if it % 2 == 0:
                    nc.vector.tensor_copy(yt, yp)
                else:
                    nc.scalar.activation(out=yt, in_=yp, func=AFT.Copy)
                if it % 2 == 0:
                    nc.sync.dma_start(out=y_out[it * 128:(it + 1) * 128, :], in_=yt)
                else:
                    nc.scalar.dma_start(out=y_out[it * 128:(it + 1) * 128, :], in_=yt)
    nc.finalize()
    return nc


def _get_nc():
    if "nc" not in _cache:
        _cache["nc"] = _build()
    return _cache["nc"]


def _run_device(inputs, trace=False):
    _cache["b_out"] = np.asarray(inputs["b_out"], np.float32)
    x = np.asarray(inputs["x"], np.float32)
    cp_mask = np.asarray(inputs["cp_mask"])
    w_qkv = np.asarray(inputs["w_qkv"], np.float32)
    w_out = np.asarray(inputs["w_out"], np.float32)

    bf = mybir.dt.np(BF16)
    maskT = np.ascontiguousarray(cp_mask.T).astype(bf)
    wqk = np.ascontiguousarray(w_qkv[:, :2 * INNER])
    wvbf = np.ascontiguousarray(w_qkv[:, 2 * INNER:]).astype(bf)
    wobf = np.ascontiguousarray(w_out).astype(bf)

    in_maps = []
    for b in range(B):
        xTb = np.ascontiguousarray(x[b].T)
        in_maps.append({
            "xT": xTb,
            "xTbf": xTb.astype(bf),
            "maskT": maskT,
            "wqk": wqk,
            "wvbf": wvbf,
            "wobf": wobf,
        })

    nc = _get_nc()
    res = run_bass_kernel_spmd(nc, in_maps, core_ids=list(range(B)), trace=trace)
    y = np.stack([res.results[b]["y"] for b in range(B)])
    strips = np.stack([res.results[b]["score"] for b in range(B)])  # [B, 8, N]
    nnz = np.count_nonzero(cp_mask, axis=1)
    score = strips.astype(np.float64).sum(axis=1) * SCALE / nnz[None, :]
    return y, score.astype(np.float32), res


def _apply_swap(y, score, patches):
    b_out = _cache.get("b_out")
    if b_out is not None:
        y = y + b_out
    idx = np.argsort(score, axis=-1, kind="stable")[::-1]
    out = y.copy()
    clone = y
    bi = np.arange(B)
    for i in range(1, patches + 1):
        ti = idx[:, i]
        out[bi, i] = clone[bi, ti]
        out[bi, ti] = clone[:, i]
    return out


def kernel(**inputs):
    patches = int(np.asarray(inputs["patches_in_core_nodes"]))
    y, score, _ = _run_device(inputs, trace=False)
    return _apply_swap(y, score, patches)
